# revision 3
# baseline (speedup 1.0000x reference)
"""ExLoss (nn_ExLoss) Trainium2 kernel.

Computes, for B=32, C=40000, D=2048:
    outputs = (inputs @ V.T) * 10
    loss    = bu_loss (CE) + h_loss (batch hard-neg) + 3 * th_loss (table hard-neg)

Sharding: V is split row-wise across 8 NeuronCores (5000 rows each). Each core
computes X @ V_shard.T with X = [10*inputs ; V[targets]] (stacked, 64 rows):
rows 0:32 are the `outputs` slice, rows 32:64 are the nsims_t slice. The
softmax sum-exp (with a fixed shift of 50 for overflow safety) and the masked
softplus sum/count of the table loss are reduced on-device; the tiny [32,32]
batch-sims loss and the final scalar assembly run on host.

The `nsims_t < 0.9999` self-similarity exclusion is applied as an exact host
correction: the only column that can exceed 0.9999 in row i is column
targets[i] (value ~1.0), so its softplus and count are subtracted after the
device pass.
"""

from contextlib import ExitStack

import ml_dtypes
import numpy as np

import concourse.bass as bass  # noqa: F401  (registers engine types)
import concourse.mybir as mybir
import concourse.tile as tile
from concourse import bacc
from concourse._compat import with_exitstack
from concourse.bass_utils import run_bass_kernel_spmd

# Problem constants (hardcoded per contract; kernel.py must be self-contained)
B, C, D = 32, 40000, 2048
T_SCALE = 10.0
P_MARGIN, N_MARGIN = 0.2, 0.3
W_BU, W_H, W_TH = 1.0, 1.0, 3.0

NCORES = 8
SHARD = C // NCORES            # 5000 bank rows per core
NB = 500                       # matmul free-dim block (one PSUM bank: <=512 f32)
NBLK = SHARD // NB             # 10 blocks per core
COH = 2                        # cohorts; 5 PSUM accumulators live at a time
NPC = NBLK // COH              # 5 blocks per cohort
CW = SHARD // COH              # 2500 columns per cohort
KT = D // 128                  # 16 contraction chunks
SHIFT = 50.0                   # exp(x - SHIFT) for overflow-safe softmax sums

MM_DT = mybir.dt.bfloat16
NP_MM_DT = ml_dtypes.bfloat16

VT_BUFS = 6
PSUM_BUFS = 7

_NC_CACHE = {}


@with_exitstack
def _device_kernel(ctx, tc, vt, xt, csts, out, stats):
    nc = tc.nc
    f32 = mybir.dt.float32
    X_AXIS = mybir.AxisListType.X

    const_pool = ctx.enter_context(tc.tile_pool(name="const", bufs=1))
    vt_pool = ctx.enter_context(tc.tile_pool(name="vt", bufs=VT_BUFS))
    psum_pool = ctx.enter_context(tc.tile_pool(name="psum", bufs=PSUM_BUFS, space="PSUM"))
    ob_pool = ctx.enter_context(tc.tile_pool(name="ob", bufs=3))
    e_pool = ctx.enter_context(tc.tile_pool(name="e", bufs=3))
    sp_pool = ctx.enter_context(tc.tile_pool(name="sp", bufs=3))
    m_pool = ctx.enter_context(tc.tile_pool(name="m", bufs=3))
    scr_pool = ctx.enter_context(tc.tile_pool(name="scr", bufs=4))

    # X.T pre-packed on host as [128, KT*64]: xts[p, k*64+m] = X[m, k*128+p]
    xts = const_pool.tile([128, KT * 64], MM_DT)
    nc.sync.dma_start(xts[:], xt[:])

    # csts cols: 0 = exp bias (-SHIFT rows 0:32, 0 rows 32:64); 1 = n_th
    # (rows 32:64); 2 = 1.0 (ln bias); 3 = unused
    cst = const_pool.tile([64, 4], f32)
    nc.sync.dma_start(cst[:], csts[:])
    bias_ap = cst[:, 0:1]
    nth_ap = cst[32:64, 1:2]
    ones_ap = cst[32:64, 2:3]

    # per-block partial reductions, one column per n-block
    s_parts = const_pool.tile([64, NBLK], f32)      # sum exp(x - SHIFT)
    th_parts = const_pool.tile([64, NBLK], f32)     # masked softplus sums
    cnt_parts = const_pool.tile([64, NBLK], f32)    # mask counts

    for coh in range(COH):
        psums = [
            psum_pool.tile([64, NB], f32, tag="acc", name=f"acc_{coh}_{j}")
            for j in range(NPC)
        ]
        for k in range(KT):
            vtile = vt_pool.tile([128, CW], MM_DT, tag="vt")
            nc.sync.dma_start(
                vtile[:], vt[k * 128:(k + 1) * 128, coh * CW:(coh + 1) * CW]
            )
            lhsT = xts[:, k * 64:(k + 1) * 64]
            for j in range(NPC):
                nc.tensor.matmul(
                    psums[j][:],
                    lhsT,
                    vtile[:, j * NB:(j + 1) * NB],
                    start=(k == 0),
                    stop=(k == KT - 1),
                )
        for j in range(NPC):
            jj = coh * NPC + j
            ps = psums[j]

            # outputs slice (rows 0:32 of the accumulator) -> DRAM
            ob = ob_pool.tile([32, NB], f32, tag="ob")
            nc.vector.tensor_copy(ob[:], ps[0:32, :])
            nc.sync.dma_start(out[:, jj * NB:(jj + 1) * NB], ob[:])

            # E = exp(x + bias); accum -> rowsum (rows 0:32 = CE sum-exp part)
            E = e_pool.tile([64, NB], f32, tag="e")
            nc.scalar.activation(
                E[:], ps[:], mybir.ActivationFunctionType.Exp,
                bias=bias_ap, scale=1.0,
                accum_out=s_parts[:, jj:jj + 1],
            )
            # softplus(nsims) = ln(1 + exp(nsims)) on rows 32:64
            SP = sp_pool.tile([64, NB], f32, tag="sp")
            nc.scalar.activation(
                SP[32:64, :], E[32:64, :], mybir.ActivationFunctionType.Ln,
                bias=ones_ap, scale=1.0,
            )
            # mask = nsims > n_th (the < 0.9999 test is host-corrected);
            # fused accum gives the mask count in the same op
            m = m_pool.tile([64, NB], f32, tag="m")
            nc.vector.tensor_scalar(
                out=m[32:64, :], in0=ps[32:64, :],
                scalar1=nth_ap, scalar2=None,
                op0=mybir.AluOpType.is_gt, op1=mybir.AluOpType.add,
                accum_out=cnt_parts[32:64, jj:jj + 1],
            )
            # masked softplus sum -> th_parts column
            spm = scr_pool.tile([64, NB], f32, tag="spm")
            nc.vector.scalar_tensor_tensor(
                out=spm[32:64, :], in0=SP[32:64, :], scalar=1.0, in1=m[32:64, :],
                op0=mybir.AluOpType.mult, op1=mybir.AluOpType.mult,
                accum_out=th_parts[32:64, jj:jj + 1],
            )

    # fold the per-block partials and ship [64, 4] stats
    stb = const_pool.tile([64, 4], f32)
    nc.vector.memset(stb[:], 0.0)
    nc.vector.reduce_sum(stb[:, 0:1], s_parts[:], axis=X_AXIS)
    nc.vector.reduce_sum(stb[32:64, 1:2], th_parts[32:64, :], axis=X_AXIS)
    nc.vector.reduce_sum(stb[32:64, 2:3], cnt_parts[32:64, :], axis=X_AXIS)
    nc.sync.dma_start(stats[:], stb[:])


def _build_nc():
    if "nc" in _NC_CACHE:
        return _NC_CACHE["nc"]
    nc = bacc.Bacc(
        "TRN2", target_bir_lowering=False, debug=False, num_devices=NCORES
    )
    f32 = mybir.dt.float32
    vt = nc.dram_tensor("vt", [D, SHARD], MM_DT, kind="ExternalInput").ap()
    xt = nc.dram_tensor("xt", [128, KT * 64], MM_DT, kind="ExternalInput").ap()
    csts = nc.dram_tensor("csts", [64, 4], f32, kind="ExternalInput").ap()
    out = nc.dram_tensor("out", [B, SHARD], f32, kind="ExternalOutput").ap()
    stats = nc.dram_tensor("stats", [64, 4], f32, kind="ExternalOutput").ap()
    with tile.TileContext(nc) as tc:
        _device_kernel(tc, vt, xt, csts, out, stats)
    nc.compile()
    _NC_CACHE["nc"] = nc
    return nc


def _prepare_inputs(inputs, V, targets):
    """Host-side shard/layout prep. Returns (in_maps, host_ctx)."""
    inputs = np.asarray(inputs, dtype=np.float32)
    V = np.asarray(V, dtype=np.float32)
    targets = np.asarray(targets).astype(np.int64)

    Vt_rows = V[targets]                                   # [B, D] f32
    X = np.concatenate([inputs * T_SCALE, Vt_rows], axis=0)  # [64, D]
    Xb = X.astype(NP_MM_DT)
    # xts[p, k*64+m] = X.T[k*128+p, m]
    xt_host = np.ascontiguousarray(
        Xb.T.reshape(KT, 128, 2 * B).transpose(1, 0, 2)
    ).reshape(128, KT * 2 * B)

    ni = inputs / np.linalg.norm(inputs.astype(np.float64), axis=1, keepdims=True)
    nth = (np.einsum("bd,bd->b", ni, Vt_rows.astype(np.float64)) - N_MARGIN)

    csts = np.zeros((2 * B, 4), dtype=np.float32)
    csts[:B, 0] = -SHIFT
    csts[B:, 1] = nth.astype(np.float32)
    csts[:, 2] = 1.0

    Vb = V.astype(NP_MM_DT)                                # [C, D] bf16
    in_maps = []
    for c in range(NCORES):
        vt_c = np.ascontiguousarray(Vb[c * SHARD:(c + 1) * SHARD, :].T)
        in_maps.append({"vt": vt_c, "xt": xt_host, "csts": csts})

    host_ctx = {
        "inputs": inputs, "V": V, "targets": targets,
        "ni": ni, "nth": nth, "Vb_rows": Vb[targets],
    }
    return in_maps, host_ctx


def _softplus(x):
    return np.logaddexp(x, 0.0)


def _masked_bce_np(logits, mask, target):
    per = target * _softplus(-logits) + (1.0 - target) * _softplus(logits)
    cnt = mask.sum()
    s = np.where(mask, per, 0.0).sum()
    return s / max(cnt, 1) if cnt > 0 else 0.0


def _h_loss_host(inputs, targets, ni):
    n = inputs.shape[0]
    sims = ni @ ni.T
    eye = np.eye(n, dtype=bool)
    same = targets[:, None] == targets[None, :]
    pos = same & ~eye
    neg = ~same
    n_thrds = np.min(np.where(pos, sims, 2.0), axis=1, keepdims=True) - N_MARGIN
    p_thrds = np.max(np.where(pos, sims, -2.0), axis=1, keepdims=True) - P_MARGIN
    hp_mask = pos & (sims < p_thrds)
    hn_mask = neg & (sims > n_thrds)
    return _masked_bce_np(sims, hp_mask, 1.0) + _masked_bce_np(sims, hn_mask, 0.0)


def _postprocess(results, host_ctx):
    targets = host_ctx["targets"]
    V = host_ctx["V"]

    outputs = np.concatenate([r["out"] for r in results], axis=1)  # [B, C] f32

    stats = [r["stats"].astype(np.float64) for r in results]
    S = sum(s[:B, 0] for s in stats)                 # sum exp(x - SHIFT) per row
    th_sum = sum(s[B:, 1] for s in stats).sum()
    cnt = sum(s[B:, 2] for s in stats).sum()

    # bu_loss: -mean log_softmax(outputs)[i, t_i]
    lse = SHIFT + np.log(S)
    x_t = outputs[np.arange(B), targets].astype(np.float64)
    bu_loss = np.mean(lse - x_t)

    # exact correction for the self-similarity column (nsims >= 0.9999):
    # in row i only column targets[i] (value ~1.0) can trip the threshold.
    vb32 = host_ctx["Vb_rows"].astype(np.float32)
    selfsim = np.einsum("bd,bd->b", vb32, vb32, dtype=np.float64)
    th_sum -= _softplus(selfsim).sum()
    cnt -= B

    th_loss = th_sum / max(cnt, 1.0) if cnt > 0 else 0.0
    active = bool(np.all(V.sum(axis=1) != 0))
    if not active:
        th_loss = 0.0

    h_loss = _h_loss_host(host_ctx["inputs"].astype(np.float64), targets,
                          host_ctx["ni"])

    loss = W_BU * bu_loss + W_H * h_loss + W_TH * th_loss
    return np.float32(loss), outputs


def _run_device(in_maps, trace=False, **kwargs):
    nc = _build_nc()
    return run_bass_kernel_spmd(nc, in_maps, list(range(NCORES)), trace=trace,
                                **kwargs)


def kernel(inputs, V, targets, label_to_pairs=None, indexs=None):
    in_maps, host_ctx = _prepare_inputs(inputs, V, targets)
    res = _run_device(in_maps, trace=False)
    return _postprocess(res.results, host_ctx)


def kernel_traced(inputs, V, targets, label_to_pairs=None, indexs=None,
                  **trace_kwargs):
    """Like kernel(), but also returns the BassKernelResults (exec_time_ns)."""
    in_maps, host_ctx = _prepare_inputs(inputs, V, targets)
    res = _run_device(in_maps, trace=True, **trace_kwargs)
    return _postprocess(res.results, host_ctx), res
